# revision 15
# baseline (speedup 1.0000x reference)
"""Row-wise argmax + label lookup kernel for Trainium2 (8 NeuronCores).

Problem: inputs (16777216, 8) f32, label_table (8,) int32.
    y[i] = label_table[argmax(inputs[i, :])]   (first-occurrence ties)

Sharding: rows split evenly across 8 cores (data parallel, no comms).
Each core streams its 64 MiB slice through SBUF in 2 MiB tiles. Per tile
the Vector engine does:
  1. a pairwise tensor_tensor max tree (8 -> 4 -> 2 -> 1) for the row max
     (reads both operands on separate ports: 2048+1024+512 cycles), and
  2. one custom DVE instruction (GROUP_ARGMAX_ANT) that encodes the
     first-occurrence argmax of every 8-element group in a single 4096-cycle
     streaming pass, using a page-stepped score scan and a global running
     max over candidate scores (pages occupy disjoint ascending score
     ranges, so the running max self-segments), and
  3. a small tensor_tensor subtract against a static iota tile to decode
     the argmax of each row from the last element of its page.
The tiny label lookup is applied on the host from the int row-argmax.
"""

import numpy as np

N_CORES = 8
ROWS = 16777216
C = 8
ROWS_PER_CORE = ROWS // N_CORES  # 2_097_152
P = 128
TILE_F = 8192  # f32 elements per partition per tile (32 KiB)
GROUPS = TILE_F // C  # 1024 rows per partition per tile
ROWS_PER_TILE = P * GROUPS  # 131_072
N_TILES = ROWS_PER_CORE // ROWS_PER_TILE  # 16

PG_STEP = 16.0  # custom-op page score step (2*C)

_NC_CACHE = {}
_REGISTERED = {}


# --------------------------------------------------------------------------
# Custom DVE op: grouped (pages of 8) first-occurrence argmax encoder.
#
#   Idx'  = scan(ADD, One, init=One)          -> g + 2  (g = global elem pos)
#   pg    = scan(ADD, One, init=C1, step=C1)  -> C1*(s+1)    (C1 = 16)
#   score = pg - Idx' = 8s + 14 - k           (k = pos in page, 0..7)
#   q     = Src0 >= Src1                      (element equals its group max)
#   cand  = q * score                         (0 when not a candidate)
#   run   = scan(MAX, cand, init=One)
#
# score is positive, strictly descending within a page, and pages occupy
# disjoint ascending ranges, so the global running max at the last element
# of page s equals the score of the FIRST group-max occurrence in page s:
#   run[s, 7] = 8s + 14 - argmax_s
# --------------------------------------------------------------------------

def _group_argmax_ref(in0, in1, s0, s1, imm2):
    a = np.asarray(in0, np.float32)
    Pp, S, N = a.shape
    b = np.broadcast_to(np.asarray(in1, np.float32), a.shape)
    s_idx = np.arange(S, dtype=np.float32)[None, :, None]
    k_idx = np.arange(N, dtype=np.float32)[None, None, :]
    idxp = s_idx * N + k_idx + 2.0
    pg = PG_STEP * (s_idx + 1.0)
    score = np.broadcast_to(pg - idxp, a.shape).astype(np.float32)
    q = (a >= b).astype(np.float32)
    cand = (q * score).astype(np.float32)
    run = np.maximum.accumulate(cand.reshape(Pp, S * N), axis=1).reshape(a.shape)
    return np.maximum(run, 1.0).astype(np.float32)  # scan init = One


def _get_group_argmax_op():
    if "op" in _REGISTERED:
        return _REGISTERED["op"]

    from concourse import dve_ops
    from concourse.dve_ops import DveOp
    from concourse.dve_spec import (
        C1, AluOp, One, Scan, Spec, Src0, Src1, lower,
    )
    from concourse.dve_uop import DveOpSpec

    name = "GROUP_ARGMAX_ANT"
    idxp = Scan(AluOp.ADD, One, init=One)
    pg = Scan(AluOp.ADD, One, init=C1, _subdim_step=C1)
    score = pg - idxp
    q = Src0 >= Src1
    cand = q * score
    # Scan.__post_init__ conservatively rejects scan-valued exprs, but the
    # scheduler resolves scheduled-Alu operands fine (each scan gets its own
    # stage with same-stage feedback). Swap the real expr in post-hoc.
    run = Scan(AluOp.MAX, One, init=One)
    object.__setattr__(run, "expr", cand)
    spec = Spec(body=run, reference=_group_argmax_ref)

    opcode = dve_ops._CUSTOM_DVE_ROW_BASE + len(dve_ops.OPS)
    assert opcode < 0x20
    dve_ops._SUB_OPCODE_FOR_NAME[name] = opcode
    shas = {}
    for ver in ("v3", "v4"):
        uops = lower(spec, ver=ver)
        shas[ver] = DveOpSpec(
            name=name, uops=uops, opcode=opcode, rd1_en=True
        ).sha(ver)
    op = DveOp(name, spec, subdim=True, uops_sha=shas)
    dve_ops.OPS.append(op)
    dve_ops.CUSTOM_DVE_SPECS[name] = spec
    _REGISTERED["op"] = op
    return op


def _build_nc(n_tiles=N_TILES):
    import concourse.tile as tile
    from concourse import bacc, mybir

    f32 = mybir.dt.float32
    i32 = mybir.dt.int32
    Alu = mybir.AluOpType
    argmax_op = _get_group_argmax_op()

    rows = n_tiles * ROWS_PER_TILE
    nc = bacc.Bacc("TRN2", target_bir_lowering=False)
    x = nc.dram_tensor("x", [rows * C], f32, kind="ExternalInput")
    y = nc.dram_tensor("y", [rows], i32, kind="ExternalOutput")
    xr = x.rearrange("(t p f) -> t p f", t=n_tiles, p=P)
    yr = y.rearrange("(t p j) -> t p j", t=n_tiles, p=P)

    with tile.TileContext(nc) as tc:
        with tc.tile_pool(name="xp", bufs=3) as xp, \
             tc.tile_pool(name="mp", bufs=2) as mp, \
             tc.tile_pool(name="op_", bufs=3) as op_, \
             tc.tile_pool(name="cst", bufs=1) as cst:
            # i87[p, j] = 8j + 14 (decode tile for the custom op's scores)
            i87 = cst.tile([P, GROUPS], f32)
            nc.gpsimd.iota(i87[:, :], [[8, GROUPS]], channel_multiplier=0,
                           allow_small_or_imprecise_dtypes=True)
            nc.vector.tensor_scalar_add(i87[:, :], i87[:, :], 14.0)

            for t in range(n_tiles):
                xt = xp.tile([P, TILE_F], f32)
                nc.gpsimd.dma_start(out=xt[:], in_=xr[t])
                x3 = xt[:].rearrange("p (j c) -> p j c", c=C)

                # pairwise max tree: 8 -> 4 -> 2 -> 1 (both DVE read ports)
                m1 = mp.tile([P, GROUPS * 4], f32, tag="m1")
                m13 = m1[:].rearrange("p (j c) -> p j c", c=4)
                nc.vector.tensor_tensor(
                    out=m13, in0=x3[:, :, 0:8:2], in1=x3[:, :, 1:8:2], op=Alu.max)
                m2 = mp.tile([P, GROUPS * 2], f32, tag="m2")
                m23 = m2[:].rearrange("p (j c) -> p j c", c=2)
                nc.vector.tensor_tensor(
                    out=m23, in0=m13[:, :, 0:4:2], in1=m13[:, :, 1:4:2], op=Alu.max)
                m = mp.tile([P, GROUPS], f32, tag="m")
                nc.vector.tensor_tensor(
                    out=m[:].unsqueeze(2), in0=m23[:, :, 0:2:2],
                    in1=m23[:, :, 1:2:2], op=Alu.max)

                # one streaming pass encodes all grouped argmaxes; the
                # result overwrites the input tile in place (the DVE write
                # of element i trails its read by the pipeline depth)
                r3 = x3
                mb = m[:].unsqueeze(2).broadcast_to([P, GROUPS, C])
                nc.vector._custom_dve(
                    argmax_op, out=r3, in0=x3, in1=mb, s0=0.0, s1=PG_STEP)

                # idx = (8j + 14) - run[:, :, 7], cast to int32 on output
                o = op_.tile([P, GROUPS], i32)
                nc.vector.tensor_tensor(
                    out=o[:].unsqueeze(2), in0=i87[:].unsqueeze(2),
                    in1=r3[:, :, 7:8], op=Alu.subtract)
                nc.gpsimd.dma_start(out=yr[t], in_=o[:])
    nc.finalize()
    return nc


def _get_nc(n_tiles=N_TILES):
    if n_tiles not in _NC_CACHE:
        _NC_CACHE[n_tiles] = _build_nc(n_tiles)
    return _NC_CACHE[n_tiles]


def kernel(inputs, label_table):
    x = np.ascontiguousarray(np.asarray(inputs, dtype=np.float32))
    lt = np.asarray(label_table)
    assert x.shape == (ROWS, C), x.shape

    from concourse.bass_utils import run_bass_kernel_spmd

    nc = _get_nc()
    in_maps = [
        {"x": x[i * ROWS_PER_CORE:(i + 1) * ROWS_PER_CORE].reshape(-1)}
        for i in range(N_CORES)
    ]
    res = run_bass_kernel_spmd(nc, in_maps, core_ids=list(range(N_CORES)))
    idx = np.concatenate([res.results[i]["y"] for i in range(N_CORES)])
    return np.take(lt, idx).astype(lt.dtype)


# revision 18
# speedup vs baseline: 1.1040x; 1.1040x over previous
"""Row-wise argmax + label lookup kernel for Trainium2 (8 NeuronCores).

Problem: inputs (16777216, 8) f32, label_table (8,) int32.
    y[i] = label_table[argmax(inputs[i, :])]   (first-occurrence ties)

Sharding: rows split evenly across 8 cores (data parallel, no comms).
Each core streams its 64 MiB slice through SBUF in 2 MiB tiles. Per tile
the Vector engine does:
  1. a pairwise tensor_tensor max tree (8 -> 4 -> 2 -> 1) for the row max
     (reads both operands on separate ports: 2048+1024+512 cycles), and
  2. one custom DVE instruction (GROUP_ARGMAX_ANT) that encodes the
     first-occurrence argmax of every 8-element group in a single 4096-cycle
     streaming pass, using a page-stepped score scan and a global running
     max over candidate scores (pages occupy disjoint ascending score
     ranges, so the running max self-segments), and
  3. a small tensor_tensor subtract against a static iota tile to decode
     the argmax of each row from the last element of its page.
The tiny label lookup is applied on the host from the int row-argmax.
"""

import numpy as np

N_CORES = 8
ROWS = 16777216
C = 8
ROWS_PER_CORE = ROWS // N_CORES  # 2_097_152
P = 128
TILE_F = 8192  # f32 elements per partition per tile (32 KiB)
GROUPS = TILE_F // C  # 1024 rows per partition per tile
ROWS_PER_TILE = P * GROUPS  # 131_072
N_TILES = ROWS_PER_CORE // ROWS_PER_TILE  # 16

PG_STEP = 16.0  # custom-op page score step (2*C)

_NC_CACHE = {}
_REGISTERED = {}


# --------------------------------------------------------------------------
# Custom DVE op: grouped (pages of 8) first-occurrence argmax encoder.
#
#   Idx'  = scan(ADD, One, init=One)          -> g + 2  (g = global elem pos)
#   pg    = scan(ADD, One, init=C1, step=C1)  -> C1*(s+1)    (C1 = 16)
#   score = pg - Idx' = 8s + 14 - k           (k = pos in page, 0..7)
#   q     = Src0 >= Src1                      (element equals its group max)
#   cand  = q * score                         (0 when not a candidate)
#   run   = scan(MAX, cand, init=One)
#
# score is positive, strictly descending within a page, and pages occupy
# disjoint ascending ranges, so the global running max at the last element
# of page s equals the score of the FIRST group-max occurrence in page s:
#   run[s, 7] = 8s + 14 - argmax_s
# --------------------------------------------------------------------------

def _group_argmax_ref(in0, in1, s0, s1, imm2):
    a = np.asarray(in0, np.float32)
    Pp, S, N = a.shape
    b = np.broadcast_to(np.asarray(in1, np.float32), a.shape)
    s_idx = np.arange(S, dtype=np.float32)[None, :, None]
    k_idx = np.arange(N, dtype=np.float32)[None, None, :]
    idxp = s_idx * N + k_idx + 2.0
    pg = PG_STEP * (s_idx + 1.0)
    score = np.broadcast_to(pg - idxp, a.shape).astype(np.float32)
    q = (a >= b).astype(np.float32)
    cand = (q * score).astype(np.float32)
    run = np.maximum.accumulate(cand.reshape(Pp, S * N), axis=1).reshape(a.shape)
    return np.maximum(run, 1.0).astype(np.float32)  # scan init = One


def _get_group_argmax_op():
    if "op" in _REGISTERED:
        return _REGISTERED["op"]

    from concourse import dve_ops
    from concourse.dve_ops import DveOp
    from concourse.dve_spec import (
        C1, AluOp, One, Scan, Spec, Src0, Src1, lower,
    )
    from concourse.dve_uop import DveOpSpec

    name = "GROUP_ARGMAX_ANT"
    idxp = Scan(AluOp.ADD, One, init=One)
    pg = Scan(AluOp.ADD, One, init=C1, _subdim_step=C1)
    score = pg - idxp
    q = Src0 >= Src1
    cand = q * score
    # Scan.__post_init__ conservatively rejects scan-valued exprs, but the
    # scheduler resolves scheduled-Alu operands fine (each scan gets its own
    # stage with same-stage feedback). Swap the real expr in post-hoc.
    run = Scan(AluOp.MAX, One, init=One)
    object.__setattr__(run, "expr", cand)
    spec = Spec(body=run, reference=_group_argmax_ref)

    opcode = dve_ops._CUSTOM_DVE_ROW_BASE + len(dve_ops.OPS)
    assert opcode < 0x20
    dve_ops._SUB_OPCODE_FOR_NAME[name] = opcode
    shas = {}
    for ver in ("v3", "v4"):
        uops = lower(spec, ver=ver)
        shas[ver] = DveOpSpec(
            name=name, uops=uops, opcode=opcode, rd1_en=True
        ).sha(ver)
    op = DveOp(name, spec, subdim=True, uops_sha=shas)
    dve_ops.OPS.append(op)
    dve_ops.CUSTOM_DVE_SPECS[name] = spec
    _REGISTERED["op"] = op
    return op


def _build_nc(n_tiles=N_TILES):
    import concourse.tile as tile
    from concourse import bacc, mybir

    f32 = mybir.dt.float32
    i32 = mybir.dt.int32
    Alu = mybir.AluOpType
    argmax_op = _get_group_argmax_op()

    rows = n_tiles * ROWS_PER_TILE
    nc = bacc.Bacc("TRN2", target_bir_lowering=False)
    x = nc.dram_tensor("x", [rows * C], f32, kind="ExternalInput")
    y = nc.dram_tensor("y", [rows], f32, kind="ExternalOutput")
    xr = x.rearrange("(t p f) -> t p f", t=n_tiles, p=P)
    yr = y.rearrange("(t p j) -> t p j", t=n_tiles, p=P)

    with tile.TileContext(nc) as tc:
        with tc.tile_pool(name="xp", bufs=3) as xp, \
             tc.tile_pool(name="mp", bufs=2) as mp, \
             tc.tile_pool(name="op_", bufs=3) as op_:
            for t in range(n_tiles):
                xt = xp.tile([P, TILE_F], f32)
                nc.gpsimd.dma_start(out=xt[:], in_=xr[t])
                x3 = xt[:].rearrange("p (j c) -> p j c", c=C)

                # pairwise max tree: 8 -> 4 -> 2 -> 1 (both DVE read ports)
                m1 = mp.tile([P, GROUPS * 4], f32, tag="m1")
                m13 = m1[:].rearrange("p (j c) -> p j c", c=4)
                nc.vector.tensor_tensor(
                    out=m13, in0=x3[:, :, 0:8:2], in1=x3[:, :, 1:8:2], op=Alu.max)
                m2 = mp.tile([P, GROUPS * 2], f32, tag="m2")
                m23 = m2[:].rearrange("p (j c) -> p j c", c=2)
                nc.vector.tensor_tensor(
                    out=m23, in0=m13[:, :, 0:4:2], in1=m13[:, :, 1:4:2], op=Alu.max)
                m = mp.tile([P, GROUPS], f32, tag="m")
                nc.vector.tensor_tensor(
                    out=m[:].unsqueeze(2), in0=m23[:, :, 0:2:2],
                    in1=m23[:, :, 1:2:2], op=Alu.max)

                # one streaming pass encodes all grouped argmaxes, written
                # COMPACT: the out AP repeats each page's address 8 times
                # (stride-0 innermost), so the last write -- the page's
                # final running max -- wins. Decoding idx = (8j+14) - run
                # happens on the host.
                runc = op_.tile([P, GROUPS], f32)
                rc3 = runc[:].unsqueeze(2).broadcast_to([P, GROUPS, C])
                mb = m[:].unsqueeze(2).broadcast_to([P, GROUPS, C])
                nc.vector._custom_dve(
                    argmax_op, out=rc3, in0=x3, in1=mb, s0=0.0, s1=PG_STEP)
                nc.gpsimd.dma_start(out=yr[t], in_=runc[:])
    nc.finalize()
    return nc


def _get_nc(n_tiles=N_TILES):
    if n_tiles not in _NC_CACHE:
        _NC_CACHE[n_tiles] = _build_nc(n_tiles)
    return _NC_CACHE[n_tiles]


def kernel(inputs, label_table):
    x = np.ascontiguousarray(np.asarray(inputs, dtype=np.float32))
    lt = np.asarray(label_table)
    assert x.shape == (ROWS, C), x.shape

    from concourse.bass_utils import run_bass_kernel_spmd

    nc = _get_nc()
    in_maps = [
        {"x": x[i * ROWS_PER_CORE:(i + 1) * ROWS_PER_CORE].reshape(-1)}
        for i in range(N_CORES)
    ]
    res = run_bass_kernel_spmd(nc, in_maps, core_ids=list(range(N_CORES)))
    runs = np.concatenate([res.results[i]["y"] for i in range(N_CORES)])
    idx = _decode(runs)
    return np.take(lt, idx).astype(lt.dtype)


def _decode(runs):
    """Decode per-row argmax from the device's score encoding."""
    r = runs.reshape(-1, GROUPS)
    j = np.arange(GROUPS, dtype=np.float32)[None, :]
    return ((C * j + 14.0) - r).astype(np.int64).reshape(-1)


# revision 19
# speedup vs baseline: 1.2307x; 1.1148x over previous
"""Row-wise argmax + label lookup kernel for Trainium2 (8 NeuronCores).

Problem: inputs (16777216, 8) f32, label_table (8,) int32.
    y[i] = label_table[argmax(inputs[i, :])]   (first-occurrence ties)

Sharding: rows split evenly across 8 cores (data parallel, no comms).
Each core streams its 64 MiB slice through SBUF in 4 MiB tiles, and a
SINGLE custom Vector-engine instruction per tile computes the
first-occurrence argmax of every 8-element row in one streaming pass:

    rm   = scan(MAX, Src1)           Src1 = x shifted one element back;
                                     a hand-built FSM step-state resets rm
                                     to -FLT_MAX at every 8-element page
                                     boundary -> exact exclusive per-row
                                     running max (raw f32 compares)
    q    = Src0 > rm                 strict increase of the row prefix max;
                                     the LAST strict increase in a row is
                                     the first occurrence of the row max
    cand = q * (scan(ADD, 1) + 1)    candidate scores = global position + 2,
                                     ascending, so the global running max
    run  = scan(MAX, cand, init=1)   self-segments across rows

run is written COMPACT (out access pattern repeats each row's address 8x;
the last write per row wins): run[row] = 8*row_in_tile + argmax + 2.
The decode and the tiny label lookup happen on the host. The first row of
each (partition, tile) block reads one uninitialized pad element in the
shifted stream; those 16K rows are recomputed exactly on the host.
"""

import numpy as np

N_CORES = 8
ROWS = 16777216
C = 8
ROWS_PER_CORE = ROWS // N_CORES  # 2_097_152
P = 128
TILE_F = 8192  # f32 elements per partition per tile (32 KiB)
GROUPS = TILE_F // C  # 1024 rows per partition per tile
ROWS_PER_TILE = P * GROUPS  # 131_072
N_TILES = ROWS_PER_CORE // ROWS_PER_TILE  # 16
PAD = 8  # pad elements before the data; keeps the DMA destination aligned

_NC_CACHE = {}
_REGISTERED = {}

FLT_MAX = float(np.finfo(np.float32).max)


def _group_argmax_ref(in0, in1, s0, s1, imm2):
    x = np.asarray(in0, np.float32)
    Pp, S, N = x.shape
    xs = np.asarray(in1, np.float32).reshape(x.shape)
    rm = np.empty_like(x)
    rm[:, 0, 0] = xs[:, 0, 0]          # page 0 k=0: steady, max(-FLT_MAX, pad)
    rm[:, 1:, 0] = -FLT_MAX            # later pages: step-state reset
    for k in range(1, N):
        rm[:, :, k] = np.maximum(rm[:, :, k - 1], xs[:, :, k])
    q = (x > rm).astype(np.float32)
    gidx = (np.arange(S * N, dtype=np.float32) + 2.0).reshape(1, S, N)
    cand = q * gidx
    run = np.maximum.accumulate(cand.reshape(Pp, S * N), axis=1).reshape(x.shape)
    return np.maximum(run, 1.0).astype(np.float32)  # run scan init = One


def _get_group_argmax_op():
    """Build + register the one-pass grouped-argmax custom DVE op.

    The framework's lower() cannot express a resetting MAX scan (subdim
    scans hold in steady state), so the 3-state FSM (seed/steady/step) is
    assembled manually and the DveOp compile cache is pre-seeded.
    """
    if "op" in _REGISTERED:
        return _REGISTERED["op"]

    from concourse import dve_ops, dve_spec as ds
    from concourse.dve_ops import DveOp, _COMPILE_CACHE
    from concourse.dve_spec import AluOp, MaxNeg, One, Scan, Spec, Src0, Src1
    from concourse.dve_uop import DveOpSpec

    name = "GROUP_ARGMAX1P_ANT"

    rm = Scan(AluOp.MAX, Src1)
    q = Src0 > rm
    idxp = Scan(AluOp.ADD, One, init=One)
    cand = q * idxp
    run = Scan(AluOp.MAX, One, init=One)
    object.__setattr__(run, "expr", cand)   # bypass conservative nesting check
    spec = Spec(body=run, reference=_group_argmax_ref)

    opcode = dve_ops._CUSTOM_DVE_ROW_BASE + len(dve_ops.OPS)
    assert opcode < 0x20
    dve_ops._SUB_OPCODE_FOR_NAME[name] = opcode

    shas = {}
    for ver in ("v3", "v4"):
        spec2 = ds._hoist_stream_invariant_ops(spec)
        scans = ds._collect(spec2.body, Scan)
        p = ds._build_placement(spec2, scans, ds.N_STAGES[ver], ds.N_LANES[ver])
        seed_ov, step_ov = ds._scan_overrides(scans, p.node_stage)
        assert not step_ov
        rm2 = [s for s in scans if s.op == AluOp.MAX and s.expr is Src1]
        assert len(rm2) == 1, scans
        my_step_ov = {p.node_stage[rm2[0]]: ds._Stage(AluOp.BYPASS, MaxNeg)}

        body_lvs = ds._body_scan_leaves(spec2)
        consume = (Src0 in body_lvs, Src1 in body_lvs)
        assert consume == (True, True)

        T = ds.Trigger
        states = [
            ds._State(placement=p, overrides=seed_ov, trigger=ds.COUNT_ONCE,
                      repeat=1, next=(1, 0, 0), write_out=False),
            ds._State(placement=p, consume=consume,
                      trigger=(T.SRC_TENSOR_DONE, T.SUB_DIM_DONE, T.NONE),
                      next=(0, 2, 0)),
            ds._State(placement=p, consume=consume, overrides=my_step_ov,
                      trigger=(T.SRC_TENSOR_DONE, T.SUB_DIM_DONE, T.COUNT),
                      next=(0, 2, 1), repeat=1),
        ]
        uops = [ds._assemble(s) for s in states]
        for u in uops:
            u.validate(ver)
        compiled = DveOpSpec(name=name, uops=uops, opcode=opcode, rd1_en=True)
        shas[ver] = compiled.sha(ver)
        _COMPILE_CACHE[(name, ver)] = compiled

    op = DveOp(name, spec, subdim=True, uops_sha=shas)
    dve_ops.OPS.append(op)
    dve_ops.CUSTOM_DVE_SPECS[name] = spec
    _REGISTERED["op"] = op
    return op


def _build_nc(n_tiles=N_TILES):
    import concourse.tile as tile
    from concourse import bacc, mybir

    f32 = mybir.dt.float32
    argmax_op = _get_group_argmax_op()

    rows = n_tiles * ROWS_PER_TILE
    nc = bacc.Bacc("TRN2", target_bir_lowering=False)
    x = nc.dram_tensor("x", [rows * C], f32, kind="ExternalInput")
    y = nc.dram_tensor("y", [rows], f32, kind="ExternalOutput")
    xr = x.rearrange("(t p f) -> t p f", t=n_tiles, p=P)
    yr = y.rearrange("(t p j) -> t p j", t=n_tiles, p=P)

    with tile.TileContext(nc) as tc:
        with tc.tile_pool(name="xp", bufs=4) as xp, \
             tc.tile_pool(name="op_", bufs=4) as op_:
            for t in range(n_tiles):
                xt = xp.tile([P, TILE_F + PAD], f32)
                nc.gpsimd.dma_start(out=xt[:, PAD:], in_=xr[t])
                x3 = xt[:, PAD:].rearrange("p (j c) -> p j c", c=C)
                xs3 = xt[:, PAD - 1:TILE_F + PAD - 1].rearrange(
                    "p (j c) -> p j c", c=C)

                runc = op_.tile([P, GROUPS], f32)
                rc3 = runc[:].unsqueeze(2).broadcast_to([P, GROUPS, C])
                nc.vector._custom_dve(
                    argmax_op, out=rc3, in0=x3, in1=xs3, s0=0.0, s1=0.0)
                nc.gpsimd.dma_start(out=yr[t], in_=runc[:])
    nc.finalize()
    return nc


def _get_nc(n_tiles=N_TILES):
    if n_tiles not in _NC_CACHE:
        _NC_CACHE[n_tiles] = _build_nc(n_tiles)
    return _NC_CACHE[n_tiles]


def _decode(runs, x2d, n_tiles=N_TILES):
    """Decode per-row argmax from the device's score encoding.

    runs: flat f32 encodings, row-major in (tile, partition, group) order.
    x2d:  the same rows' raw inputs, for the exact host recompute of each
          (partition, tile) block's first row (its shifted stream read one
          uninitialized pad element on device).
    """
    r = runs.reshape(-1, GROUPS)
    j = np.arange(GROUPS, dtype=np.float32)[None, :]
    idx = ((r - 2.0) - C * j).astype(np.int64)
    first_rows = (np.arange(r.shape[0]) * GROUPS)  # row ids of j == 0
    idx[:, 0] = np.argmax(x2d[first_rows], axis=1)
    return idx.reshape(-1)


def kernel(inputs, label_table):
    x = np.ascontiguousarray(np.asarray(inputs, dtype=np.float32))
    lt = np.asarray(label_table)
    assert x.shape == (ROWS, C), x.shape

    from concourse.bass_utils import run_bass_kernel_spmd

    nc = _get_nc()
    in_maps = [
        {"x": x[i * ROWS_PER_CORE:(i + 1) * ROWS_PER_CORE].reshape(-1)}
        for i in range(N_CORES)
    ]
    res = run_bass_kernel_spmd(nc, in_maps, core_ids=list(range(N_CORES)))
    runs = np.concatenate([res.results[i]["y"] for i in range(N_CORES)])
    idx = _decode(runs, x)
    return np.take(lt, idx).astype(lt.dtype)


# revision 23
# speedup vs baseline: 1.5163x; 1.2321x over previous
"""Row-wise argmax + label lookup kernel for Trainium2 (8 NeuronCores).

Problem: inputs (16777216, 8) f32, label_table (8,) int32.
    y[i] = label_table[argmax(inputs[i, :])]   (first-occurrence ties)

Sharding: rows split evenly across 8 cores (data parallel, no comms).
Each core streams its 64 MiB slice through SBUF in 4 MiB tiles, and a
SINGLE custom Vector-engine instruction per tile computes the
first-occurrence argmax of every 8-element row in one streaming pass:

    rm   = scan(MAX, Src1)           Src1 = x shifted one element back;
                                     a hand-built FSM step-state resets rm
                                     to -FLT_MAX at every 8-element page
                                     boundary -> exact exclusive per-row
                                     running max (raw f32 compares)
    q    = Src0 > rm                 strict increase of the row prefix max;
                                     the LAST strict increase in a row is
                                     the first occurrence of the row max
    cand = q * (scan(ADD, 1) + 1)    candidate scores = global position + 2,
                                     ascending, so the global running max
    run  = scan(MAX, cand, init=1)   self-segments across rows

run is written COMPACT (out access pattern repeats each row's address 8x;
the last write per row wins): run[row] = 8*row_in_tile + argmax + 2.
The decode and the tiny label lookup happen on the host. The first row of
each (partition, tile) block reads one uninitialized pad element in the
shifted stream; those 16K rows are recomputed exactly on the host.
"""

import numpy as np

N_CORES = 8
ROWS = 16777216
C = 8
ROWS_PER_CORE = ROWS // N_CORES  # 2_097_152
P = 128
TILE_F = 8192  # f32 elements per partition per tile (32 KiB)
GROUPS = TILE_F // C  # 1024 rows per partition per tile
ROWS_PER_TILE = P * GROUPS  # 131_072
N_TILES = ROWS_PER_CORE // ROWS_PER_TILE  # 16
PAD = 8  # pad elements before the data; keeps the DMA destination aligned

_NC_CACHE = {}
_REGISTERED = {}

FLT_MAX = float(np.finfo(np.float32).max)


def _group_argmax_ref(in0, in1, s0, s1, imm2):
    x = np.asarray(in0, np.float32)
    Pp, S, N = x.shape
    xs = np.asarray(in1, np.float32).reshape(x.shape)
    rm = np.empty_like(x)
    rm[:, 0, 0] = xs[:, 0, 0]          # page 0 k=0: steady, max(-FLT_MAX, pad)
    rm[:, 1:, 0] = -FLT_MAX            # later pages: step-state reset
    for k in range(1, N):
        rm[:, :, k] = np.maximum(rm[:, :, k - 1], xs[:, :, k])
    q = (x > rm).astype(np.float32)
    gidx = (np.arange(S * N, dtype=np.float32) + 2.0).reshape(1, S, N)
    cand = q * gidx
    run = np.maximum.accumulate(cand.reshape(Pp, S * N), axis=1).reshape(x.shape)
    return np.maximum(run, 1.0).astype(np.float32)  # run scan init = One


def _get_group_argmax_op():
    """Build + register the one-pass grouped-argmax custom DVE op.

    The framework's lower() cannot express a resetting MAX scan (subdim
    scans hold in steady state), so the 3-state FSM (seed/steady/step) is
    assembled manually and the DveOp compile cache is pre-seeded.
    """
    if "op" in _REGISTERED:
        return _REGISTERED["op"]

    from concourse import dve_ops, dve_spec as ds
    from concourse.dve_ops import DveOp, _COMPILE_CACHE
    from concourse.dve_spec import AluOp, MaxNeg, One, Scan, Spec, Src0, Src1
    from concourse.dve_uop import DveOpSpec

    name = "GROUP_ARGMAX1P_ANT"

    rm = Scan(AluOp.MAX, Src1)
    q = Src0 > rm
    idxp = Scan(AluOp.ADD, One, init=One)
    cand = q * idxp
    run = Scan(AluOp.MAX, One, init=One)
    object.__setattr__(run, "expr", cand)   # bypass conservative nesting check
    spec = Spec(body=run, reference=_group_argmax_ref)

    opcode = dve_ops._CUSTOM_DVE_ROW_BASE + len(dve_ops.OPS)
    assert opcode < 0x20
    dve_ops._SUB_OPCODE_FOR_NAME[name] = opcode

    shas = {}
    for ver in ("v3", "v4"):
        spec2 = ds._hoist_stream_invariant_ops(spec)
        scans = ds._collect(spec2.body, Scan)
        p = ds._build_placement(spec2, scans, ds.N_STAGES[ver], ds.N_LANES[ver])
        seed_ov, step_ov = ds._scan_overrides(scans, p.node_stage)
        assert not step_ov
        rm2 = [s for s in scans if s.op == AluOp.MAX and s.expr is Src1]
        assert len(rm2) == 1, scans
        my_step_ov = {p.node_stage[rm2[0]]: ds._Stage(AluOp.BYPASS, MaxNeg)}

        body_lvs = ds._body_scan_leaves(spec2)
        consume = (Src0 in body_lvs, Src1 in body_lvs)
        assert consume == (True, True)

        T = ds.Trigger
        states = [
            ds._State(placement=p, overrides=seed_ov, trigger=ds.COUNT_ONCE,
                      repeat=1, next=(1, 0, 0), write_out=False),
            ds._State(placement=p, consume=consume,
                      trigger=(T.SRC_TENSOR_DONE, T.SUB_DIM_DONE, T.NONE),
                      next=(0, 2, 0)),
            ds._State(placement=p, consume=consume, overrides=my_step_ov,
                      trigger=(T.SRC_TENSOR_DONE, T.SUB_DIM_DONE, T.COUNT),
                      next=(0, 2, 1), repeat=1),
        ]
        uops = [ds._assemble(s) for s in states]
        for u in uops:
            u.validate(ver)
        compiled = DveOpSpec(name=name, uops=uops, opcode=opcode, rd1_en=True)
        shas[ver] = compiled.sha(ver)
        _COMPILE_CACHE[(name, ver)] = compiled

    op = DveOp(name, spec, subdim=True, uops_sha=shas)
    dve_ops.OPS.append(op)
    dve_ops.CUSTOM_DVE_SPECS[name] = spec
    _REGISTERED["op"] = op
    return op


SPLIT = 4  # the first tile is split for a faster pipeline ramp


def _tile_plan(n_tiles):
    """List of (elems_per_partition, groups_per_partition) per device tile."""
    plan = [(TILE_F // SPLIT, GROUPS // SPLIT)] * SPLIT
    plan += [(TILE_F, GROUPS)] * (n_tiles - 1)
    return plan


def _build_nc(n_tiles=N_TILES):
    import concourse.tile as tile
    from concourse import bacc, mybir

    f32 = mybir.dt.float32
    u16 = mybir.dt.uint16
    argmax_op = _get_group_argmax_op()

    rows = n_tiles * ROWS_PER_TILE
    nc = bacc.Bacc("TRN2", target_bir_lowering=False)
    x = nc.dram_tensor("x", [rows * C], f32, kind="ExternalInput")
    y = nc.dram_tensor("y", [rows], u16, kind="ExternalOutput")

    plan = _tile_plan(n_tiles)
    BUFS = 4

    with tile.TileContext(nc) as tc:
        with tc.tile_pool(name="xq", bufs=BUFS) as xq, \
             tc.tile_pool(name="xp", bufs=BUFS) as xp, \
             tc.tile_pool(name="oq", bufs=BUFS) as oq, \
             tc.tile_pool(name="op_", bufs=BUFS) as op_:
            xoff = 0
            yoff = 0
            nsmall = 0
            nbig = 0
            for tf, tg in plan:
                small = tf != TILE_F
                pool, opool = (xq, oq) if small else (xp, op_)
                xt = pool.tile([P, tf + PAD], f32)
                # first use of each pool slot: initialize the one pad element
                # the shifted stream reads (keeps CoreSim's checker happy; the
                # affected rows are recomputed on the host regardless)
                cnt = nsmall if small else nbig
                if cnt < BUFS:
                    nc.vector.memset(xt[:, PAD - 1:PAD], 0.0)
                if small:
                    nsmall += 1
                else:
                    nbig += 1

                xin = x[xoff:xoff + P * tf].rearrange("(p f) -> p f", p=P)
                nc.gpsimd.dma_start(out=xt[:, PAD:], in_=xin)
                x3 = xt[:, PAD:].rearrange("p (j c) -> p j c", c=C)
                xs3 = xt[:, PAD - 1:tf + PAD - 1].rearrange(
                    "p (j c) -> p j c", c=C)

                runc = opool.tile([P, tg], u16)
                rc3 = runc[:].unsqueeze(2).broadcast_to([P, tg, C])
                nc.vector._custom_dve(
                    argmax_op, out=rc3, in0=x3, in1=xs3, s0=0.0, s1=0.0)
                yout = y[yoff:yoff + P * tg].rearrange("(p j) -> p j", p=P)
                nc.gpsimd.dma_start(out=yout, in_=runc[:])
                xoff += P * tf
                yoff += P * tg
    nc.finalize()
    return nc


def _get_nc(n_tiles=N_TILES):
    if n_tiles not in _NC_CACHE:
        _NC_CACHE[n_tiles] = _build_nc(n_tiles)
    return _NC_CACHE[n_tiles]


def _decode_core(runs_core, x_core, n_tiles=N_TILES):
    """Decode one core's argmaxes from the device's u16 score encoding.

    runs_core: (rows_per_core,) u16, row-major in (tile, partition, group)
    order. The first row of every (tile, partition) block is recomputed
    from x_core (its shifted stream read one pad element on device).
    """
    out = np.empty(runs_core.shape[0], np.int64)
    off = 0
    for tf, tg in _tile_plan(n_tiles):
        n = P * tg
        r = runs_core[off:off + n].reshape(P, tg).astype(np.int64)
        j = np.arange(tg, dtype=np.int64)[None, :]
        idx = (r - 2) - C * j
        first = off + np.arange(P) * tg  # row ids with j == 0
        idx[:, 0] = np.argmax(x_core[first], axis=1)
        out[off:off + n] = idx.reshape(-1)
        off += n
    return out


def kernel(inputs, label_table):
    x = np.ascontiguousarray(np.asarray(inputs, dtype=np.float32))
    lt = np.asarray(label_table)
    assert x.shape == (ROWS, C), x.shape

    from concourse.bass_utils import run_bass_kernel_spmd

    nc = _get_nc()
    in_maps = [
        {"x": x[i * ROWS_PER_CORE:(i + 1) * ROWS_PER_CORE].reshape(-1)}
        for i in range(N_CORES)
    ]
    res = run_bass_kernel_spmd(nc, in_maps, core_ids=list(range(N_CORES)))
    idx = np.concatenate([
        _decode_core(
            np.asarray(res.results[i]["y"]).reshape(-1),
            x[i * ROWS_PER_CORE:(i + 1) * ROWS_PER_CORE],
        )
        for i in range(N_CORES)
    ])
    return np.take(lt, idx).astype(lt.dtype)
